# revision 2
# baseline (speedup 1.0000x reference)
"""Trainium2 Bass kernel for nn_MCLoss (scatter_memory forward).

Computes logits = inputs @ memory.T  ([4096, 2048] @ [2048, 50000] -> [4096, 50000] f32).

Strategy (tensor-parallel, per sharding hint): the memory bank is sharded
row-wise across 8 NeuronCores (exactly 6250 identity rows each). Each core
computes its [4096, 6250] logits slice with a tiled PE matmul; the host
concatenates the 8 slices.

Mixed-precision split-K (the speedup over the plain bf16 kernel): the PE runs
fp8 (e4m3) matmuls at 2x the bf16 rate via DoubleRow perf mode (contraction
256/instr at 0.5 cycles/row). Full fp8 would be ~3.7% rel err (gate 2e-2), so
the 16 k-tiles (contraction 2048 = 16x128) are split: 8 k-tiles in bf16 and 8
k-tiles in fp8 where the stationary operand (inputs) is split hi+lo into the
two DoubleRow slots (x ~ fp8(x) + fp8(x - fp8(x)), moving operand broadcast
into both slots with a stride-0 AP). The x side then contributes ~0.1% error,
so only memory's fp8 rounding counts: total rel err ~1.9e-2*0.5^0.5 ~ 1.9e-2
... measured 1.87e-2 vs the 2e-2 gate. Cost per output column: 8*1 + 8*0.5 =
12 rows vs 16 bf16 -> 1.33x fewer PE cycles.

Scaling: both operands are pre-scaled by 64 (power of two, exact in bf16; puts
unit-norm-row elements ~N(0, 0.022) in e4m3's normal range). All 16 k-tiles
share the 64*64 = 4096 chain scale in PSUM; eviction does tensor_scalar_mul by
2^-12 with bf16 downcast (output stored bf16, upcast on host).

Layout per core (identical SPMD program):
  - lhs stationary, resident in SBUF the whole kernel: bf16 tiles
    [128, 32m, 8i, 128] (even k-tiles) + fp8 hi/lo tiles [128, 32m, 8i, 2, 128]
    (odd k-tiles); 128 KiB/partition total.
  - rhs moving, streamed per column group: bf16 [128, 8i, w] + fp8 [128, 8i, w]
    (slot dim supplied by broadcast_to, no duplication).
  - Column groups 2x309 + 11x512 = 6250; per (group, m): 8 interleaved pairs
    (bf16 matmul w cycles, then fp8 DoubleRow w/2 cycles) accumulating into one
    PSUM bank, so each instruction's weight load hides under the previous
    instruction's moving stream.

kernel._build(reps=N) emits the compute body N times (idempotent writes) so
test.py can measure marginal per-rep device time with dispatch overhead
cancelled.
"""
import numpy as np
import ml_dtypes

import concourse.bass as bass
import concourse.mybir as mybir
import concourse.tile as tile
from concourse import bacc
from concourse.bass_utils import run_bass_kernel_spmd

P = 128
B = 4096          # rows of inputs
D = 2048          # features (contraction)
C = 50000         # memory rows (classes)
N_CORES = 8
N_SHARD = C // N_CORES          # 6250 per-core logits width (exact)
MT = B // P                     # 32 m-tiles
KT = D // P                     # 16 k-tiles
NPAIR = KT // 2                 # 8 (bf16, fp8) k-tile pairs
SCALE = 64.0                    # operand pre-scale (2^6)
DESCALE = 1.0 / (SCALE * SCALE)

# fp8 DoubleRow chunk width cap (rhs free = 2*w per instr). 512 first; flip to
# 256 if the NEFF compiler rejects rhs free > 512.
DR_W_MAX = 512

# Column groups: two 309-wide first (cheap first tile -> earlier PE start),
# then eleven 512-wide. 2*309 + 11*512 = 6250.
GROUPS = []
_c0 = 0
for _w in [309, 309] + [512] * 11:
    GROUPS.append((_c0, _w))
    _c0 += _w
assert _c0 == N_SHARD

_NC_CACHE = {}


def _build(reps=1):
    """Build the SPMD program. reps>1 repeats the whole compute body (same
    inputs -> same outputs, idempotent) so test.py can measure the marginal
    per-rep device time with launch overhead cancelled out."""
    if reps in _NC_CACHE:
        return _NC_CACHE[reps]
    bf = mybir.dt.bfloat16
    f8 = mybir.dt.float8e4
    nc = bacc.Bacc("TRN2", target_bir_lowering=False, debug=False)
    lhs_bf = nc.dram_tensor("lhs_bf", [P, MT, NPAIR, P], bf, kind="ExternalInput")
    lhs_f8 = nc.dram_tensor("lhs_f8", [P, MT, NPAIR, 2, P], f8, kind="ExternalInput")
    rhs_bf = nc.dram_tensor("rhs_bf", [NPAIR * P, N_SHARD], bf, kind="ExternalInput")
    rhs_f8 = nc.dram_tensor("rhs_f8", [NPAIR * P, N_SHARD], f8, kind="ExternalInput")
    out = nc.dram_tensor("out", [B, N_SHARD], bf, kind="ExternalOutput")
    rbf_r = rhs_bf[:].rearrange("(i p) c -> p i c", p=P)
    rf8_r = rhs_f8[:].rearrange("(i p) c -> p i c", p=P)

    with tile.TileContext(nc) as tc:
        with (
            tc.tile_pool(name="rhsp", bufs=2) as rhsp,
            tc.tile_pool(name="lhsp", bufs=2 * MT) as lhsp,
            tc.tile_pool(name="outp", bufs=4) as outp,
            tc.tile_pool(name="psump", bufs=6, space="PSUM") as psump,
        ):
            # First group's rhs, then the whole lhs (resident for the kernel).
            c0_0, w0 = GROUPS[0]
            rb = rhsp.tile([P, NPAIR, w0], bf, tag="rbf")
            rf = rhsp.tile([P, NPAIR, w0], f8, tag="rf8")
            nc.sync.dma_start(out=rb[:], in_=rbf_r[:, :, c0_0 : c0_0 + w0])
            nc.sync.dma_start(out=rf[:], in_=rf8_r[:, :, c0_0 : c0_0 + w0])
            lbs, lfs = [], []
            for m in range(MT):
                lb = lhsp.tile([P, NPAIR, P], bf, tag="lbf")
                lf = lhsp.tile([P, NPAIR, 2, P], f8, tag="lf8")
                nc.sync.dma_start(out=lb[:], in_=lhs_bf[:, m, :, :])
                nc.sync.dma_start(out=lf[:], in_=lhs_f8[:, m, :, :, :])
                lbs.append(lb)
                lfs.append(lf)

            for rep in range(reps):
                for gi, (c0, w) in enumerate(GROUPS):
                    if gi > 0 or rep > 0:
                        rb = rhsp.tile([P, NPAIR, w], bf, tag="rbf")
                        rf = rhsp.tile([P, NPAIR, w], f8, tag="rf8")
                        nc.sync.dma_start(out=rb[:], in_=rbf_r[:, :, c0 : c0 + w])
                        nc.sync.dma_start(out=rf[:], in_=rf8_r[:, :, c0 : c0 + w])
                    # fp8 DoubleRow chunking within the group
                    chunks = []
                    cc = 0
                    while cc < w:
                        cw = min(DR_W_MAX, w - cc)
                        chunks.append((cc, cw))
                        cc += cw
                    for m in range(MT):
                        ps = psump.tile([P, w], mybir.dt.float32, tag="ps")
                        for i in range(NPAIR):
                            nc.tensor.matmul(
                                ps[:],
                                lhsT=lbs[m][:, i, :],
                                rhs=rb[:, i, :],
                                start=(i == 0),
                                stop=False,
                            )
                            for ci, (cc0, cw) in enumerate(chunks):
                                nc.tensor.matmul(
                                    ps[:, cc0 : cc0 + cw],
                                    lhsT=lfs[m][:, i, :, :],
                                    rhs=rf[:, i, cc0 : cc0 + cw]
                                    .unsqueeze(1)
                                    .broadcast_to([P, 2, cw]),
                                    start=False,
                                    stop=(i == NPAIR - 1 and ci == len(chunks) - 1),
                                    perf_mode=mybir.MatmulPerfMode.DoubleRow,
                                )
                        ot = outp.tile([P, w], bf, tag="out")
                        nc.vector.tensor_scalar_mul(ot[:], ps[:], DESCALE)
                        nc.scalar.dma_start(
                            out=out[m * P : (m + 1) * P, c0 : c0 + w], in_=ot[:]
                        )
    nc.compile()
    _NC_CACHE[reps] = nc
    return nc


def _prep_inputs(inputs, memory):
    f8 = ml_dtypes.float8_e4m3
    bf = ml_dtypes.bfloat16
    X = np.asarray(inputs, dtype=np.float32) * SCALE          # [B, D]
    Xr = X.reshape(MT, P, KT, P)                              # [m, j, k, p]
    # bf16 half: even k-tiles. lhs_bf[p, m, i, j] = X[m*128+j, (2i)*128+p]
    lhs_bf = np.ascontiguousarray(
        Xr[:, :, 0::2, :].transpose(3, 0, 2, 1).astype(bf)
    )
    # fp8 half: odd k-tiles, hi+lo split.
    T = np.ascontiguousarray(Xr[:, :, 1::2, :].transpose(3, 0, 2, 1))  # [p, m, i, j]
    hi = T.astype(f8)
    lo = (T - hi.astype(np.float32)).astype(f8)
    lhs_f8 = np.ascontiguousarray(np.stack([hi, lo], axis=3))  # [p, m, i, 2, j]

    M = np.asarray(memory, dtype=np.float32) * SCALE          # [C, D]
    Msh = M.reshape(N_CORES, N_SHARD, KT, P)                  # [core, c, k, p]
    # rhs_bf[core, i*128+p, c] = M[c_global, (2i)*128+p]
    rhs_bf = np.ascontiguousarray(
        Msh[:, :, 0::2, :].transpose(0, 2, 3, 1).astype(bf)
    ).reshape(N_CORES, NPAIR * P, N_SHARD)
    rhs_f8 = np.ascontiguousarray(
        Msh[:, :, 1::2, :].transpose(0, 2, 3, 1).astype(f8)
    ).reshape(N_CORES, NPAIR * P, N_SHARD)
    return lhs_bf, lhs_f8, rhs_bf, rhs_f8


def kernel(inputs, targets, memory):
    """Full-input entry point: returns logits [4096, 50000] float32."""
    nc = _build()
    lhs_bf, lhs_f8, rhs_bf, rhs_f8 = _prep_inputs(inputs, memory)
    in_maps = [
        {
            "lhs_bf": lhs_bf,
            "lhs_f8": lhs_f8,
            "rhs_bf": rhs_bf[c],
            "rhs_f8": rhs_f8[c],
        }
        for c in range(N_CORES)
    ]
    res = run_bass_kernel_spmd(nc, in_maps, core_ids=list(range(N_CORES)))
    logits = np.concatenate(
        [res.results[c]["out"].astype(np.float32) for c in range(N_CORES)], axis=1
    )
    return np.ascontiguousarray(logits)
